# revision 14
# baseline (speedup 1.0000x reference)
"""PointPillarScatter on 8 Trainium2 NeuronCores.

out[b, c, y*NX+x] = pillar_features[p, c] for each pillar p with coords
(b, 0, y, x); duplicate (b,y,x) resolved last-pillar-wins; rest zeros.

Sharding: data-parallel over batch - core b handles batch element b.

Per-core device algorithm (selection-matmul formulation of the scatter):
  The BEV plane [C=64, S=214272] is split into 432 windows of W=496 slots
  (496 f32 = one PSUM bank).  Windows are processed in pairs (t, t+216)
  packed block-diagonally so one matmul emits both:

    lhsT[128, 128]: rows 0:64   = window t      pillars, feature cols 0:64
                    rows 64:128 = window t+216  pillars, feature cols 64:128
    Sel[k, s] = (idxloc[k] == s)   one-hot over the window's 496 slots
    psum[128, 496] = lhsT^T @ Sel  -> partitions 0:64 = window t [C, 496],
                                      partitions 64:128 = window t+216

  fp32 matmul with a one-hot rhs is exact (each output column has a single
  1.0 term).  ACT copies PSUM->SBUF; stores write [64, 1984] contiguous
  slabs per output half.  Host prep dedups (last-wins, matching the
  reference scatter), buckets pillars by window (max occupancy 58 < 64),
  and packs the block-diagonal table.

  Measured alternatives this replaces: GPSIMD ap_gather assembly (~29 ns
  per slot-column -> ~3 ms/core) and unpaired [64, 496] matmuls (~400 us,
  PE- and ACT-bound at M=64 = half the PE array).
"""

import numpy as np

NX, NY = 432, 496
C = 64
S = NY * NX            # 214272
W = 496                # slots per window = one PSUM bank of f32
NWIN = S // W          # 432
NPAIR = NWIN // 2      # 216 block-diagonal pairs: (t, t+216)
KH = 64                # max pillars per window (half of the 128 k rows)
TROWS = NPAIR * 128    # 27648 table rows
B = 8

_cache = {}


def _build_program():
    import concourse.bacc as bacc
    import concourse.tile as tile
    import concourse.mybir as mybir

    dt = mybir.dt
    nc = bacc.Bacc("TRN2", target_bir_lowering=False, debug=False, num_devices=B)

    feats = nc.dram_tensor("feats", [TROWS, 2 * C], dt.float32, kind="ExternalInput")
    idxloc = nc.dram_tensor("idxloc", [128, NPAIR], dt.float32, kind="ExternalInput")
    iota = nc.dram_tensor("iota", [128, W], dt.float32, kind="ExternalInput")
    out = nc.dram_tensor("out", [C, S], dt.float32, kind="ExternalOutput")

    LB = 8   # pairs of lhsT per staging load
    SG = 4   # pairs per store stage: [64, 4*496] slabs per half

    with tile.TileContext(nc) as tc:
        with (
            tc.tile_pool(name="const", bufs=1) as cpool,
            tc.tile_pool(name="lhs", bufs=3) as lpool,
            tc.tile_pool(name="sel", bufs=6) as selpool,
            tc.tile_pool(name="psum", bufs=8, space="PSUM") as ppool,
            tc.tile_pool(name="stg", bufs=3) as spool,
        ):
            iot = cpool.tile([128, W], dt.float32)
            nc.sync.dma_start(out=iot[:], in_=iota.ap())
            idxt = cpool.tile([128, NPAIR], dt.float32)
            nc.sync.dma_start(out=idxt[:], in_=idxloc.ap())

            feats_b = feats.ap().rearrange(
                "(g n p) c -> g p n c", g=NPAIR // LB, p=128
            )

            lt = None
            stg = None
            for t in range(NPAIR):
                if t % LB == 0:
                    lt = lpool.tile([128, LB * 2 * C], dt.float32, tag="lt")
                    nc.sync.dma_start(
                        out=lt[:].rearrange("p (n c) -> p n c", c=2 * C),
                        in_=feats_b[t // LB],
                    )
                n = t % LB
                sel = selpool.tile([128, W], dt.float32, tag="sel")
                nc.vector.tensor_tensor(
                    out=sel[:],
                    in0=iot[:],
                    in1=idxt[:, t : t + 1].to_broadcast([128, W]),
                    op=mybir.AluOpType.is_equal,
                )
                pt = ppool.tile([128, W], dt.float32, tag="pt")
                nc.tensor.matmul(
                    out=pt[:],
                    lhsT=lt[:, n * 2 * C : (n + 1) * 2 * C],
                    rhs=sel[:],
                    start=True,
                    stop=True,
                )
                u = t % SG
                if u == 0:
                    stg = spool.tile([128, SG * W], dt.float32, tag="stg")
                cp = nc.scalar.copy if t % 2 == 0 else nc.vector.tensor_copy
                cp(out=stg[:, u * W : (u + 1) * W], in_=pt[:])
                if u == SG - 1:
                    t0 = t - (SG - 1)
                    nc.sync.dma_start(
                        out=out.ap()[:, t0 * W : (t0 + SG) * W], in_=stg[0:64, :]
                    )
                    nc.sync.dma_start(
                        out=out.ap()[:, (NPAIR + t0) * W : (NPAIR + t0 + SG) * W],
                        in_=stg[64:128, :],
                    )

    nc.compile()
    return nc


def _get_program():
    if "nc" not in _cache:
        _cache["nc"] = _build_program()
    return _cache["nc"]


def _host_prep(pillar_features, coords_b, coords_z, coords_y, coords_x):
    """Shard pillars by batch; dedup last-wins; pack block-diagonal pairs."""
    feats = np.ascontiguousarray(np.asarray(pillar_features, dtype=np.float32))
    cb = np.asarray(coords_b).astype(np.int64)
    idx = (
        np.asarray(coords_z).astype(np.int64)
        + np.asarray(coords_y).astype(np.int64) * NX
        + np.asarray(coords_x).astype(np.int64)
    )

    iota = np.broadcast_to(np.arange(W, dtype=np.float32), (128, W)).copy()

    in_maps = []
    for b in range(B):
        sel = np.nonzero(cb == b)[0]
        idx_b = idx[sel]
        # keep last occurrence per flat index (reference scatter semantics)
        rev = idx_b[::-1]
        uniq, pos_rev = np.unique(rev, return_index=True)
        keep = sel[len(idx_b) - 1 - pos_rev]  # pillar ids, sorted by idx

        wbin = uniq // W                       # window id, sorted
        woff = (uniq % W).astype(np.float32)
        counts = np.bincount(wbin, minlength=NWIN)
        assert counts.max() <= KH, f"batch {b}: window overflow {counts.max()}"
        starts = np.zeros(NWIN, np.int64)
        starts[1:] = np.cumsum(counts)[:-1]
        rank = np.arange(len(uniq)) - starts[wbin]
        # window w pairs into block t = w % NPAIR, half h = w // NPAIR
        blk = wbin % NPAIR
        half = wbin // NPAIR
        rows = 128 * blk + KH * half + rank    # table row of each pillar
        fp = np.zeros((TROWS, 2 * C), dtype=np.float32)
        fp[rows[:, None], (half * C)[:, None] + np.arange(C)[None, :]] = feats[keep]
        il = np.full((NPAIR, 128), -1.0, dtype=np.float32)
        il[blk, KH * half + rank] = woff
        in_maps.append({"feats": fp, "idxloc": il.T.copy(), "iota": iota})
    return in_maps


def kernel(pillar_features, coords_b, coords_z, coords_y, coords_x, batch_size):
    from concourse.bass_utils import run_bass_kernel_spmd

    assert int(batch_size) == B
    nc = _get_program()
    in_maps = _host_prep(pillar_features, coords_b, coords_z, coords_y, coords_x)
    res = run_bass_kernel_spmd(nc, in_maps, core_ids=list(range(B)), trace=False)
    out = np.empty((B, C, NY, NX), dtype=np.float32)
    for b in range(B):
        out[b] = res.results[b]["out"].reshape(C, NY, NX)
    return out


# revision 16
# speedup vs baseline: 1.0359x; 1.0359x over previous
"""PointPillarScatter on 8 Trainium2 NeuronCores.

out[b, c, y*NX+x] = pillar_features[p, c] for each pillar p with coords
(b, 0, y, x); duplicate (b,y,x) resolved last-pillar-wins; rest zeros.

Sharding: data-parallel over batch - core b handles batch element b.

Per-core device algorithm (selection-matmul formulation of the scatter):
  The BEV plane [C=64, S=214272] is split into 432 windows of W=496 slots
  (496 f32 = one PSUM bank).  Windows are processed in pairs (t, t+216)
  packed block-diagonally so one matmul emits both:

    lhsT[128, 128]: rows 0:64   = window t      pillars, feature cols 0:64
                    rows 64:128 = window t+216  pillars, feature cols 64:128
    Sel[k, s] = (idxloc[k] == s)   one-hot over the window's 496 slots
    psum[128, 496] = lhsT^T @ Sel  -> partitions 0:64 = window t [C, 496],
                                      partitions 64:128 = window t+216

  fp32 matmul with a one-hot rhs is exact (each output column has a single
  1.0 term).  ACT copies PSUM->SBUF; stores write [64, 1984] contiguous
  slabs per output half.  Host prep dedups (last-wins, matching the
  reference scatter), buckets pillars by window (max occupancy 58 < 64),
  and packs the block-diagonal table.

  Measured alternatives this replaces: GPSIMD ap_gather assembly (~29 ns
  per slot-column -> ~3 ms/core) and unpaired [64, 496] matmuls (~400 us,
  PE- and ACT-bound at M=64 = half the PE array).
"""

import numpy as np

NX, NY = 432, 496
C = 64
S = NY * NX            # 214272
W = 496                # slots per window = one PSUM bank of f32
NWIN = S // W          # 432
NPAIR = NWIN // 2      # 216 block-diagonal pairs: (t, t+216)
KH = 64                # max pillars per window (half of the 128 k rows)
TROWS = NPAIR * 128    # 27648 table rows
B = 8

_cache = {}


def _build_program():
    import concourse.bacc as bacc
    import concourse.tile as tile
    import concourse.mybir as mybir

    dt = mybir.dt
    nc = bacc.Bacc("TRN2", target_bir_lowering=False, debug=False, num_devices=B)

    feats = nc.dram_tensor("feats", [TROWS, 2 * C], dt.float32, kind="ExternalInput")
    idxloc = nc.dram_tensor("idxloc", [128, NPAIR], dt.float32, kind="ExternalInput")
    iota = nc.dram_tensor("iota", [128, W], dt.float32, kind="ExternalInput")
    out = nc.dram_tensor("out", [C, S], dt.float32, kind="ExternalOutput")

    LB = 8   # pairs of lhsT per staging load
    SG = 4   # pairs per store stage: [64, 4*496] slabs per half

    with tile.TileContext(nc) as tc:
        with (
            tc.tile_pool(name="const", bufs=1) as cpool,
            tc.tile_pool(name="lhs", bufs=3) as lpool,
            tc.tile_pool(name="sel", bufs=6) as selpool,
            tc.tile_pool(name="psum", bufs=4, space="PSUM") as ppool,
            tc.tile_pool(name="stg", bufs=3) as spool,
        ):
            iot = cpool.tile([128, W], dt.float32)
            nc.sync.dma_start(out=iot[:], in_=iota.ap())
            idxt = cpool.tile([128, NPAIR], dt.float32)
            nc.sync.dma_start(out=idxt[:], in_=idxloc.ap())

            feats_b = feats.ap().rearrange(
                "(g n p) c -> g p n c", g=NPAIR // LB, p=128
            )

            WP = 512  # padded window pitch: PSUM-bank-aligned matmul targets
            lt = None
            stg = None
            pt = None
            for t in range(NPAIR):
                if t % LB == 0:
                    lt = lpool.tile([128, LB * 2 * C], dt.float32, tag="lt")
                    nc.sync.dma_start(
                        out=lt[:].rearrange("p (n c) -> p n c", c=2 * C),
                        in_=feats_b[t // LB],
                    )
                n = t % LB
                sel = selpool.tile([128, W], dt.float32, tag="sel")
                nc.vector.tensor_tensor(
                    out=sel[:],
                    in0=iot[:],
                    in1=idxt[:, t : t + 1].to_broadcast([128, W]),
                    op=mybir.AluOpType.is_equal,
                )
                v = t % 2
                if v == 0:
                    pt = ppool.tile([128, 2 * WP], dt.float32, tag="pt")
                nc.tensor.matmul(
                    out=pt[:, v * WP : v * WP + W],
                    lhsT=lt[:, n * 2 * C : (n + 1) * 2 * C],
                    rhs=sel[:],
                    start=True,
                    stop=True,
                )
                u = t % SG
                if u == 0:
                    stg = spool.tile([128, SG * WP], dt.float32, tag="stg")
                if v == 1:
                    nc.scalar.copy(
                        out=stg[:, (u - 1) * WP : (u + 1) * WP], in_=pt[:]
                    )
                if u == SG - 1:
                    t0 = t - (SG - 1)
                    sb = stg[:, :].rearrange("p (j s) -> p j s", s=WP)[:, :, 0:W]
                    dr = out.ap().rearrange("c (j s) -> c j s", s=W)
                    nc.sync.dma_start(
                        out=dr[:, t0 : t0 + SG], in_=sb[0:64]
                    )
                    nc.sync.dma_start(
                        out=dr[:, NPAIR + t0 : NPAIR + t0 + SG], in_=sb[64:128]
                    )

    nc.compile()
    return nc


def _get_program():
    if "nc" not in _cache:
        _cache["nc"] = _build_program()
    return _cache["nc"]


def _host_prep(pillar_features, coords_b, coords_z, coords_y, coords_x):
    """Shard pillars by batch; dedup last-wins; pack block-diagonal pairs."""
    feats = np.ascontiguousarray(np.asarray(pillar_features, dtype=np.float32))
    cb = np.asarray(coords_b).astype(np.int64)
    idx = (
        np.asarray(coords_z).astype(np.int64)
        + np.asarray(coords_y).astype(np.int64) * NX
        + np.asarray(coords_x).astype(np.int64)
    )

    iota = np.broadcast_to(np.arange(W, dtype=np.float32), (128, W)).copy()

    in_maps = []
    for b in range(B):
        sel = np.nonzero(cb == b)[0]
        idx_b = idx[sel]
        # keep last occurrence per flat index (reference scatter semantics)
        rev = idx_b[::-1]
        uniq, pos_rev = np.unique(rev, return_index=True)
        keep = sel[len(idx_b) - 1 - pos_rev]  # pillar ids, sorted by idx

        wbin = uniq // W                       # window id, sorted
        woff = (uniq % W).astype(np.float32)
        counts = np.bincount(wbin, minlength=NWIN)
        assert counts.max() <= KH, f"batch {b}: window overflow {counts.max()}"
        starts = np.zeros(NWIN, np.int64)
        starts[1:] = np.cumsum(counts)[:-1]
        rank = np.arange(len(uniq)) - starts[wbin]
        # window w pairs into block t = w % NPAIR, half h = w // NPAIR
        blk = wbin % NPAIR
        half = wbin // NPAIR
        rows = 128 * blk + KH * half + rank    # table row of each pillar
        fp = np.zeros((TROWS, 2 * C), dtype=np.float32)
        fp[rows[:, None], (half * C)[:, None] + np.arange(C)[None, :]] = feats[keep]
        il = np.full((NPAIR, 128), -1.0, dtype=np.float32)
        il[blk, KH * half + rank] = woff
        in_maps.append({"feats": fp, "idxloc": il.T.copy(), "iota": iota})
    return in_maps


def kernel(pillar_features, coords_b, coords_z, coords_y, coords_x, batch_size):
    from concourse.bass_utils import run_bass_kernel_spmd

    assert int(batch_size) == B
    nc = _get_program()
    in_maps = _host_prep(pillar_features, coords_b, coords_z, coords_y, coords_x)
    res = run_bass_kernel_spmd(nc, in_maps, core_ids=list(range(B)), trace=False)
    out = np.empty((B, C, NY, NX), dtype=np.float32)
    for b in range(B):
        out[b] = res.results[b]["out"].reshape(C, NY, NX)
    return out
